# revision 14
# baseline (speedup 1.0000x reference)
"""Trainium2 Bass kernel for nn_Attention2 (8-head encoder/decoder attention mix).

Reference computation (full batch B=4096):
    enc_h  = relu(encoder_input @ W_enc + b_enc)               [B, 1024]
    heads  = relu(einsum('bh,khd->kbd', enc_h, W_heads) + b_heads)  [8, B, 1024]
    dec_H  = relu(decoder_input @ W_dec + b_dec)               [B, 1024]
    scores = sum(heads * dec_H, axis=2)                        [8, B]
    attn   = softmax(scores.T, axis=1)                         [B, 8]
    out    = einsum('kbd,bk->bd', heads, attn)                 [B, 1024]

Sharding: pure data-parallel over batch across 8 NeuronCores (B_loc = 512
per core, params replicated, zero collectives).

v5 design (evolved from the v1 all-bias-matmul kernel via trace analysis):
  - Half the bias-injection matmuls removed from the PE stream (644 MMs:
    608 real + 36 bias vs v1's 680). Per [128,1024] output tile, the n=0
    chunk gets its bias from a DVE tensor_tensor add (PSUM + broadcast-bias
    SBUF tile) and the n=1 chunk from a K=128 matmul of ones/128 against
    the same broadcast tile (then relu straight from PSUM). This hybrid
    keeps BOTH engines under the PE budget per head-batch-tile
    (PE 17 MM = 3.67 us; DVE = tt-add 690 + score stt 1224 + out tt-add
    690 = 2.6 us; ScalarE = 2 relu + exp + scale-copy = 2.5 us) - the
    full-DVE-bias variant measured DVE == PE and drained a ~20 us tail.
  - out_acc path: head_s = ACT(head, Copy, scale=e_h) on ScalarE (bf16),
    out_acc += head_s via all-bf16 tensor_tensor add (2x DVE mode, 690 ns
    vs 1226 stt). h==0 initializes out_acc directly (no memsets).
  - Host repacks weights so every big load is one contiguous DMA:
    W_heads -> [H][128, 8*1024] (one 2 MB DMA per head, 16 KB rows),
    W_enc -> wave-split quarters, x_encT halves, x_dec/W_dec one DMA each.
    ~28 DMA issues total, spread over the sync/scalar/gpsimd queues by
    need-time priority.
  - Stage C (dec) b-tiles are interleaved between the two stage-A waves so
    the PE has work while the second half of the stage-A inputs lands.
  - ~10 warmup matmuls on constants right after the preamble warm the HAM
    clock gate (cold PE runs 1.2 GHz for its first ~3.4 us) during the
    initial DMA wait.
  - Streaming normalizer-free softmax: e = exp(score - 24) (scores
    measured in [14, 34]); divide by sum(e) at the end.

Measured v4 milestones (FAST clock regime): MM stream at the 216 ns
roofline spacing, first MM at 7.5 us.
"""

import os
import numpy as np
from contextlib import ExitStack

N_CORES = 8
ENC_DIM, DEC_DIM, HID, HEADS, BATCH = 1024, 512, 1024, 8, 4096
B_LOC = BATCH // N_CORES          # 512 batch rows per core
P = 128                           # SBUF partitions
NCHUNK = 512                      # matmul moving free-dim (one PSUM bank)
SCORE_SHIFT = 24.0                # scores measured in [14.2, 34.0]

_cache = {}


def _build():
    import concourse.tile as tile
    from concourse import bacc, mybir

    f32 = mybir.dt.float32
    bf16 = mybir.dt.bfloat16
    MM = bf16
    ST = f32                      # head storage dtype (score stt is f32-fast)
    Relu = mybir.ActivationFunctionType.Relu
    Exp = mybir.ActivationFunctionType.Exp
    Copy = mybir.ActivationFunctionType.Copy
    X = mybir.AxisListType.X
    mult = mybir.AluOpType.mult
    add = mybir.AluOpType.add

    KT_E = ENC_DIM // P           # 8 contraction tiles (enc dim)
    KT_H = HID // P               # 8 contraction tiles (hid dim)
    KT_D = DEC_DIM // P           # 4 contraction tiles (dec dim)
    MT = HID // P                 # 8 hid tiles (feature-major partitions)
    BT = B_LOC // P               # 4 batch tiles
    NC_H = HID // NCHUNK          # 2 moving chunks over hid
    HALF = HID // 2               # 512

    N_WARMUP = int(os.environ.get("BASS_WARMUP", "10"))

    nc = bacc.Bacc("TRN2", target_bir_lowering=False, debug=False,
                   num_devices=N_CORES)

    # host-repacked inputs (see build_in_maps)
    xe_r = nc.dram_tensor("x_enc_r", [2, P, (KT_E // 2) * B_LOC], MM,
                          kind="ExternalInput").ap()
    we_r = nc.dram_tensor("w_enc_r", [2, 2, P, (KT_E // 2) * HALF], MM,
                          kind="ExternalInput").ap()
    xd_r = nc.dram_tensor("x_dec_r", [P, KT_D * B_LOC], MM,
                          kind="ExternalInput").ap()
    wd_r = nc.dram_tensor("w_dec_r", [P, KT_D * HID], MM,
                          kind="ExternalInput").ap()
    wh_r = nc.dram_tensor("w_heads_r", [HEADS, P, KT_H * HID], MM,
                          kind="ExternalInput").ap()
    b_enc_pp = nc.dram_tensor("b_enc_pp", [P, MT], f32, kind="ExternalInput").ap()
    # broadcast bias tiles: bias replicated across the 128 partitions
    b_heads_bc = nc.dram_tensor("b_heads_bc", [HEADS, P, HID], MM,
                                kind="ExternalInput").ap()
    b_dec_bc = nc.dram_tensor("b_dec_bc", [P, HID], MM, kind="ExternalInput").ap()
    out_d = nc.dram_tensor("out", [B_LOC, HID], f32, kind="ExternalOutput").ap()

    with tile.TileContext(nc) as tc, ExitStack() as ctx:
        persist = ctx.enter_context(tc.tile_pool(name="persist", bufs=1))
        psums = ctx.enter_context(tc.tile_pool(name="psums", bufs=4, space="PSUM"))

        # --- constants / biases ---
        # ones/128 so a K=128 matmul against the full broadcast-bias tile
        # sums to exactly the bias
        ones_128 = persist.tile([P, P], MM, tag="ones128", name="ones128")
        nc.vector.memset(ones_128[:], 1.0 / P)
        warm_rhs = persist.tile([P, NCHUNK], MM, tag="wrhs", name="wrhs")
        nc.vector.memset(warm_rhs[:], 0.5)
        negC = persist.tile([P, 1], f32, tag="negC", name="negC")
        nc.vector.memset(negC[:], -SCORE_SHIFT)
        benc = persist.tile([P, MT], f32, tag="benc", name="benc")
        bh_bc = [persist.tile([P, HID], MM, tag=f"bhb{h}", name=f"bhb{h}")
                 for h in range(HEADS)]
        bd_bc = persist.tile([P, HID], MM, tag="bdb", name="bdb")

        # --- persistent activations ---
        ench = [persist.tile([P, B_LOC], MM, tag=f"ench{m}", name=f"ench{m}") for m in range(MT)]
        dec_bm = [persist.tile([P, HID], ST, tag=f"dec{b}", name=f"dec{b}") for b in range(BT)]
        e_all = [persist.tile([P, HEADS], f32, tag=f"eall{b}", name=f"eall{b}") for b in range(BT)]
        out_acc = [persist.tile([P, HID], MM, tag=f"oacc{b}", name=f"oacc{b}") for b in range(BT)]

        # ---- PE warmup: matmuls on constants so HAM un-throttles while the
        # first input DMAs are still in flight. Output never read.
        warm_ps = psums.tile([P, HID], f32, tag="mm", name="warm")
        for _ in range(N_WARMUP):
            nc.tensor.matmul(warm_ps[:, :NCHUNK], ones_128[:], warm_rhs[:],
                             start=True, stop=True)

        # head-weight tiles: one [128, 8*1024] tile per head, 3 in flight
        wh_pool = ctx.enter_context(tc.tile_pool(name="wh", bufs=3))

        with ExitStack() as actx:
            a_pool = actx.enter_context(tc.tile_pool(name="stageA", bufs=1))
            # xe[half][128, 4*512], we[wave][half][128, 4*512]
            xe = [a_pool.tile([P, (KT_E // 2) * B_LOC], MM, tag=f"xe{i}", name=f"xe{i}")
                  for i in range(2)]
            we = [[a_pool.tile([P, (KT_E // 2) * HALF], MM, tag=f"we{w}{i}", name=f"we{w}{i}")
                   for i in range(2)] for w in range(2)]
            xd = a_pool.tile([P, KT_D * B_LOC], MM, tag="xd", name="xd")
            wd = a_pool.tile([P, KT_D * HID], MM, tag="wd", name="wd")

            # ---- DMA issues, by queue and need-time priority ----
            # scalar queue: x_encT halves + enc bias
            nc.scalar.dma_start(xe[0][:], xe_r[0])
            nc.scalar.dma_start(xe[1][:], xe_r[1])
            nc.scalar.dma_start(benc[:], b_enc_pp[:])
            # sync queue: W_enc wave quarters, head weights, small biases
            nc.sync.dma_start(we[0][0][:], we_r[0, 0])
            nc.sync.dma_start(we[0][1][:], we_r[0, 1])
            nc.sync.dma_start(we[1][0][:], we_r[1, 0])
            nc.sync.dma_start(we[1][1][:], we_r[1, 1])
            wh_tiles = [wh_pool.tile([P, KT_H * HID], MM, tag="whs", name=f"wh{h}")
                        for h in range(HEADS)]
            nc.sync.dma_start(wh_tiles[0][:], wh_r[0])
            nc.sync.dma_start(wh_tiles[1][:], wh_r[1])
            nc.sync.dma_start(bd_bc[:], b_dec_bc[:])
            nc.sync.dma_start(bh_bc[0][:], b_heads_bc[0])
            nc.sync.dma_start(bh_bc[1][:], b_heads_bc[1])
            for h in range(2, HEADS):
                nc.sync.dma_start(wh_tiles[h][:], wh_r[h])
            # gpsimd queue: dec inputs + remaining head bias tiles
            nc.gpsimd.dma_start(xd[:], xd_r[:])
            nc.gpsimd.dma_start(wd[:], wd_r[:])
            for h in range(2, HEADS):
                nc.gpsimd.dma_start(bh_bc[h][:], b_heads_bc[h])

            # ---- Stage C tile: hybrid bias (n=0 DVE tt-add, n=1 bias MM) --
            def stage_c_tile(b):
                ps = psums.tile([P, HID], f32, tag="mm", name="ps")
                for k in range(KT_D):
                    nc.tensor.matmul(
                        ps[:, :NCHUNK],
                        xd[:, k * B_LOC + b * P:k * B_LOC + (b + 1) * P],
                        wd[:, k * HID:k * HID + NCHUNK],
                        start=(k == 0), stop=(k == KT_D - 1))
                nc.tensor.matmul(ps[:, NCHUNK:], ones_128[:], bd_bc[:, NCHUNK:],
                                 start=True, stop=False)
                for k in range(KT_D):
                    nc.tensor.matmul(
                        ps[:, NCHUNK:],
                        xd[:, k * B_LOC + b * P:k * B_LOC + (b + 1) * P],
                        wd[:, k * HID + NCHUNK:(k + 1) * HID],
                        start=False, stop=(k == KT_D - 1))
                nc.vector.tensor_tensor(dec_bm[b][:, :NCHUNK], ps[:, :NCHUNK],
                                        bd_bc[:, :NCHUNK], op=add)
                nc.scalar.activation(dec_bm[b][:, :NCHUNK],
                                     dec_bm[b][:, :NCHUNK], Relu)
                nc.scalar.activation(dec_bm[b][:, NCHUNK:], ps[:, NCHUNK:], Relu)

            # ---- Stage A (enc trunk, feature-major), k-outer in 2 waves,
            # with stage-C b-tiles interleaved so the PE has work while the
            # second half of the stage-A inputs is still landing.
            for wave in range(2):
                pss = [psums.tile([P, HID], f32, tag="mm", name="ps")
                       for _ in range(MT // 2)]
                for k in range(KT_E):
                    xek = xe[k // 4][:, (k % 4) * B_LOC:(k % 4 + 1) * B_LOC]
                    wek = we[wave][k // 4][:, (k % 4) * HALF:(k % 4 + 1) * HALF]
                    for j in range(MT // 2):
                        nc.tensor.matmul(pss[j][:, :B_LOC],
                                         wek[:, j * P:(j + 1) * P], xek,
                                         start=(k == 0), stop=(k == KT_E - 1))
                for j in range(MT // 2):
                    m = wave * (MT // 2) + j
                    nc.scalar.activation(ench[m][:], pss[j][:, :B_LOC], Relu,
                                         bias=benc[:, m:m + 1], scale=1.0)
                for b in ((0, 1) if wave == 0 else (2, 3)):
                    stage_c_tile(b)

        # ---- Stage B + D: heads (batch-major), streaming softmax ----
        head_pool = ctx.enter_context(tc.tile_pool(name="head", bufs=3))
        scratch = ctx.enter_context(tc.tile_pool(name="scratch", bufs=4))

        for h in range(HEADS):
            wh = wh_tiles[h]
            last = h == HEADS - 1
            for b in range(BT):
                head_t = head_pool.tile([P, HID], ST, tag=f"head{b}", name=f"head{b}")
                head_s = (head_pool.tile([P, HID], MM, tag=f"hs{b}", name=f"hs{b}")
                          if h > 0 else None)
                ps = psums.tile([P, HID], f32, tag="mm", name="ps")
                # n=0 chunk: plain matmuls, bias added by DVE afterwards
                for k in range(KT_H):
                    nc.tensor.matmul(
                        ps[:, :NCHUNK], ench[k][:, b * P:(b + 1) * P],
                        wh[:, k * HID:k * HID + NCHUNK],
                        start=(k == 0), stop=(k == KT_H - 1))
                # n=1 chunk: bias injected by a K=128 matmul (ones/128 against
                # the broadcast-bias tile), relu straight from PSUM
                nc.tensor.matmul(ps[:, NCHUNK:], ones_128[:],
                                 bh_bc[h][:, NCHUNK:], start=True, stop=False)
                for k in range(KT_H):
                    nc.tensor.matmul(
                        ps[:, NCHUNK:], ench[k][:, b * P:(b + 1) * P],
                        wh[:, k * HID + NCHUNK:(k + 1) * HID],
                        start=False, stop=(k == KT_H - 1))
                prod = scratch.tile([P, HID], ST, tag="prod", name="prod")
                s_col = scratch.tile([P, 1], f32, tag="scol", name="scol")
                # n=0: tt-add + in-place relu; n=1: relu from psum
                nc.vector.tensor_tensor(head_t[:, :NCHUNK], ps[:, :NCHUNK],
                                        bh_bc[h][:, :NCHUNK], op=add)
                nc.scalar.activation(head_t[:, :NCHUNK], head_t[:, :NCHUNK],
                                     Relu)
                nc.scalar.activation(head_t[:, NCHUNK:], ps[:, NCHUNK:], Relu)
                # score: s_col = sum_hid(head * dec)  (fused mult+accum)
                if not last:
                    nc.vector.scalar_tensor_tensor(
                        prod[:], head_t[:], 1.0, dec_bm[b][:],
                        op0=mult, op1=mult, accum_out=s_col[:])
                else:
                    # last head: half-tile ops so the kernel tail pipelines
                    s_half = scratch.tile([P, 1], f32, tag="shalf", name="shalf")
                    nc.vector.scalar_tensor_tensor(
                        prod[:, :NCHUNK], head_t[:, :NCHUNK], 1.0,
                        dec_bm[b][:, :NCHUNK], op0=mult, op1=mult,
                        accum_out=s_half[:])
                    nc.vector.scalar_tensor_tensor(
                        prod[:, NCHUNK:], head_t[:, NCHUNK:], 1.0,
                        dec_bm[b][:, NCHUNK:], op0=mult, op1=mult,
                        accum_out=s_col[:])
                    nc.vector.tensor_add(s_col[:], s_col[:], s_half[:])
                # e = exp(score - C)
                nc.scalar.activation(e_all[b][:, h:h + 1], s_col[:], Exp,
                                     bias=negC[:], scale=1.0)
                # head_s = e_h * head (ScalarE scale-copy, bf16 out), then
                # out_acc += head_s on DVE (all-bf16 tensor_tensor, 2x mode).
                # h==0 writes out_acc directly (no memset, no add).
                if h == 0:
                    nc.scalar.activation(out_acc[b][:], head_t[:], Copy,
                                         scale=e_all[b][:, h:h + 1])
                elif not last:
                    nc.scalar.activation(head_s[:], head_t[:], Copy,
                                         scale=e_all[b][:, h:h + 1])
                    nc.vector.tensor_add(out_acc[b][:], out_acc[b][:], head_s[:])
                else:
                    for n in range(NC_H):
                        ncol = slice(n * NCHUNK, (n + 1) * NCHUNK)
                        nc.scalar.activation(head_s[:, ncol], head_t[:, ncol],
                                             Copy, scale=e_all[b][:, h:h + 1])
                        nc.vector.tensor_add(out_acc[b][:, ncol],
                                             out_acc[b][:, ncol],
                                             head_s[:, ncol])

        # ---- Final: divide by sum of exps, write out ----
        fin = ctx.enter_context(tc.tile_pool(name="fin", bufs=2))
        for b in range(BT):
            s_sum = fin.tile([P, 1], f32, tag="ssum", name="ssum")
            rinv = fin.tile([P, 1], f32, tag="rinv", name="rinv")
            nc.vector.reduce_sum(s_sum[:], e_all[b][:], axis=X)
            nc.vector.reciprocal(rinv[:], s_sum[:])
            out_f = fin.tile([P, HID], f32, tag="outf", name="outf")
            nc.vector.tensor_scalar_mul(out_f[:], out_acc[b][:], rinv[:])
            nc.sync.dma_start(out_d[b * P:(b + 1) * P, :], out_f[:])

    nc.compile()
    return nc


def _get_nc():
    if "nc" not in _cache:
        _cache["nc"] = _build()
    return _cache["nc"]


def build_in_maps(encoder_input, decoder_input, W_enc, b_enc, W_heads,
                  b_heads, W_dec, b_dec):
    import ml_dtypes
    bf = ml_dtypes.bfloat16
    f32c = lambda a: np.asarray(a, dtype=np.float32)
    cast = lambda a: np.ascontiguousarray(a, dtype=np.float32).astype(bf)

    KT_E, KT_D, KT_H = ENC_DIM // P, DEC_DIM // P, HID // P

    xeT = f32c(encoder_input).T                     # [1024, 4096]
    xdT = f32c(decoder_input).T                     # [512, 4096]
    W_enc = f32c(W_enc)                             # [1024, 1024]
    W_dec = f32c(W_dec)                             # [512, 1024]
    W_heads = f32c(W_heads)                         # [8, 1024, 1024]

    # w_enc_r[w, i][p, k4*512 + c] = W_enc[(i*4+k4)*128+p, w*512 + c]
    we4 = W_enc.reshape(KT_E, P, 2, HID // 2)       # [k, p, wave, c]
    we_r = np.zeros((2, 2, P, (KT_E // 2) * (HID // 2)), np.float32)
    for w in range(2):
        for i in range(2):
            blk = we4[i * 4:(i + 1) * 4, :, w, :]   # [4k, 128, 512]
            we_r[w, i] = blk.transpose(1, 0, 2).reshape(P, -1)
    xe4 = xeT.reshape(KT_E, P, BATCH)
    wd4 = W_dec.reshape(KT_D, P, HID)
    wd_r = np.ascontiguousarray(wd4.transpose(1, 0, 2).reshape(P, -1))
    wh4 = W_heads.reshape(HEADS, KT_H, P, HID)
    wh_r = np.ascontiguousarray(wh4.transpose(0, 2, 1, 3).reshape(HEADS, P, -1))

    bh_bc = np.broadcast_to(f32c(b_heads)[:, None, :], (HEADS, P, HID))
    bd_bc = np.broadcast_to(f32c(b_dec)[None, :], (P, HID))
    shared = {
        "w_enc_r": cast(we_r),
        "b_enc_pp": np.ascontiguousarray(f32c(b_enc).reshape(HID // P, P).T),
        "w_heads_r": cast(wh_r),
        "b_heads_bc": cast(bh_bc),
        "w_dec_r": cast(wd_r),
        "b_dec_bc": cast(bd_bc),
    }
    xd4 = xdT.reshape(KT_D, P, BATCH)
    in_maps = []
    for c in range(N_CORES):
        sl = slice(c * B_LOC, (c + 1) * B_LOC)
        m = dict(shared)
        xe_c = xe4[:, :, sl]                        # [8k, 128, 512]
        m["x_enc_r"] = cast(np.stack(
            [xe_c[i * 4:(i + 1) * 4].transpose(1, 0, 2).reshape(P, -1)
             for i in range(2)]))
        m["x_dec_r"] = cast(xd4[:, :, sl].transpose(1, 0, 2).reshape(P, -1))
        in_maps.append(m)
    return in_maps


def kernel(encoder_input, decoder_input, W_enc, b_enc, W_heads, b_heads,
           W_dec, b_dec):
    from concourse.bass_utils import run_bass_kernel_spmd

    nc = _get_nc()
    in_maps = build_in_maps(encoder_input, decoder_input, W_enc, b_enc,
                            W_heads, b_heads, W_dec, b_dec)
    res = run_bass_kernel_spmd(nc, in_maps, list(range(N_CORES)))
    out = np.concatenate([res.results[c]["out"] for c in range(N_CORES)], axis=0)
    return out.astype(np.float32)


# revision 20
# speedup vs baseline: 1.0772x; 1.0772x over previous
"""Trainium2 Bass kernel for nn_Attention2 (8-head encoder/decoder attention mix).

Reference computation (full batch B=4096):
    enc_h  = relu(encoder_input @ W_enc + b_enc)               [B, 1024]
    heads  = relu(einsum('bh,khd->kbd', enc_h, W_heads) + b_heads)  [8, B, 1024]
    dec_H  = relu(decoder_input @ W_dec + b_dec)               [B, 1024]
    scores = sum(heads * dec_H, axis=2)                        [8, B]
    attn   = softmax(scores.T, axis=1)                         [B, 8]
    out    = einsum('kbd,bk->bd', heads, attn)                 [B, 1024]

Sharding: pure data-parallel over batch across 8 NeuronCores (B_loc = 512
per core, params replicated, zero collectives).

v5 design (evolved from the v1 all-bias-matmul kernel via trace analysis):
  - Half the bias-injection matmuls removed from the PE stream (644 MMs:
    608 real + 36 bias vs v1's 680). Per [128,1024] output tile, the n=0
    chunk gets its bias from a DVE tensor_tensor add (PSUM + broadcast-bias
    SBUF tile) and the n=1 chunk from a K=128 matmul of ones/128 against
    the same broadcast tile (then relu straight from PSUM). This hybrid
    keeps BOTH engines under the PE budget per head-batch-tile
    (PE 17 MM = 3.67 us; DVE = tt-add 690 + score stt 1224 + out tt-add
    690 = 2.6 us; ScalarE = 2 relu + exp + scale-copy = 2.5 us) - the
    full-DVE-bias variant measured DVE == PE and drained a ~20 us tail.
  - out_acc path: head_s = ACT(head, Copy, scale=e_h) on ScalarE (bf16),
    out_acc += head_s via all-bf16 tensor_tensor add (2x DVE mode, 690 ns
    vs 1226 stt). h==0 initializes out_acc directly (no memsets).
  - Host repacks weights so every big load is one contiguous DMA:
    W_heads -> [H][128, 8*1024] (one 2 MB DMA per head, 16 KB rows),
    W_enc -> wave-split quarters, x_encT halves, x_dec/W_dec one DMA each.
    ~28 DMA issues total, spread over the sync/scalar/gpsimd queues by
    need-time priority.
  - Stage C (dec) b-tiles are interleaved between the two stage-A waves so
    the PE has work while the second half of the stage-A inputs lands.
  - ~10 warmup matmuls on constants right after the preamble warm the HAM
    clock gate (cold PE runs 1.2 GHz for its first ~3.4 us) during the
    initial DMA wait.
  - Streaming normalizer-free softmax: e = exp(score - 24) (scores
    measured in [14, 34]); divide by sum(e) at the end.

Measured v4 milestones (FAST clock regime): MM stream at the 216 ns
roofline spacing, first MM at 7.5 us.
"""

import os
import numpy as np
from contextlib import ExitStack

N_CORES = 8
ENC_DIM, DEC_DIM, HID, HEADS, BATCH = 1024, 512, 1024, 8, 4096
B_LOC = BATCH // N_CORES          # 512 batch rows per core
P = 128                           # SBUF partitions
NCHUNK = 512                      # matmul moving free-dim (one PSUM bank)
SCORE_SHIFT = 24.0                # scores measured in [14.2, 34.0]

_cache = {}


def _build():
    import concourse.tile as tile
    from concourse import bacc, mybir

    f32 = mybir.dt.float32
    bf16 = mybir.dt.bfloat16
    MM = bf16
    ST = f32                      # head storage dtype (score stt is f32-fast)
    Relu = mybir.ActivationFunctionType.Relu
    Exp = mybir.ActivationFunctionType.Exp
    Copy = mybir.ActivationFunctionType.Copy
    X = mybir.AxisListType.X
    mult = mybir.AluOpType.mult
    add = mybir.AluOpType.add

    KT_E = ENC_DIM // P           # 8 contraction tiles (enc dim)
    KT_H = HID // P               # 8 contraction tiles (hid dim)
    KT_D = DEC_DIM // P           # 4 contraction tiles (dec dim)
    MT = HID // P                 # 8 hid tiles (feature-major partitions)
    BT = B_LOC // P               # 4 batch tiles
    NC_H = HID // NCHUNK          # 2 moving chunks over hid
    HALF = HID // 2               # 512

    N_WARMUP = int(os.environ.get("BASS_WARMUP", "10"))

    nc = bacc.Bacc("TRN2", target_bir_lowering=False, debug=False,
                   num_devices=N_CORES)

    # host-repacked inputs (see build_in_maps)
    xe_r = nc.dram_tensor("x_enc_r", [2, P, (KT_E // 2) * B_LOC], MM,
                          kind="ExternalInput").ap()
    we_r = nc.dram_tensor("w_enc_r", [2, 2, P, (KT_E // 2) * HALF], MM,
                          kind="ExternalInput").ap()
    xd_r = nc.dram_tensor("x_dec_r", [P, KT_D * B_LOC], MM,
                          kind="ExternalInput").ap()
    wd_r = nc.dram_tensor("w_dec_r", [P, KT_D * HID], MM,
                          kind="ExternalInput").ap()
    wh_r = nc.dram_tensor("w_heads_r", [HEADS, P, KT_H * HID], MM,
                          kind="ExternalInput").ap()
    b_enc_pp = nc.dram_tensor("b_enc_pp", [P, MT], f32, kind="ExternalInput").ap()
    # broadcast bias tiles: bias replicated across the 128 partitions
    b_heads_bc = nc.dram_tensor("b_heads_bc", [HEADS, P, HID], MM,
                                kind="ExternalInput").ap()
    b_dec_bc = nc.dram_tensor("b_dec_bc", [P, HID], MM, kind="ExternalInput").ap()
    out_d = nc.dram_tensor("out", [B_LOC, HID], f32, kind="ExternalOutput").ap()

    with tile.TileContext(nc) as tc, ExitStack() as ctx:
        persist = ctx.enter_context(tc.tile_pool(name="persist", bufs=1))
        psums = ctx.enter_context(tc.tile_pool(name="psums", bufs=4, space="PSUM"))

        # --- constants / biases ---
        # ones/128 so a K=128 matmul against the full broadcast-bias tile
        # sums to exactly the bias
        ones_128 = persist.tile([P, P], MM, tag="ones128", name="ones128")
        nc.vector.memset(ones_128[:], 1.0 / P)
        warm_rhs = persist.tile([P, NCHUNK], MM, tag="wrhs", name="wrhs")
        nc.vector.memset(warm_rhs[:], 0.5)
        negC = persist.tile([P, 1], f32, tag="negC", name="negC")
        nc.vector.memset(negC[:], -SCORE_SHIFT)
        benc = persist.tile([P, MT], f32, tag="benc", name="benc")
        bh_bc = [persist.tile([P, HID], MM, tag=f"bhb{h}", name=f"bhb{h}")
                 for h in range(HEADS)]
        bd_bc = persist.tile([P, HID], MM, tag="bdb", name="bdb")

        # --- persistent activations ---
        ench = [persist.tile([P, B_LOC], MM, tag=f"ench{m}", name=f"ench{m}") for m in range(MT)]
        dec_bm = [persist.tile([P, HID], ST, tag=f"dec{b}", name=f"dec{b}") for b in range(BT)]
        e_all = [persist.tile([P, HEADS], f32, tag=f"eall{b}", name=f"eall{b}") for b in range(BT)]
        out_acc = [persist.tile([P, HID], ST, tag=f"oacc{b}", name=f"oacc{b}") for b in range(BT)]

        # ---- PE warmup: matmuls on constants so HAM un-throttles while the
        # first input DMAs are still in flight. Output never read.
        warm_ps = psums.tile([P, HID], f32, tag="mm", name="warm")
        for _ in range(N_WARMUP):
            nc.tensor.matmul(warm_ps[:, :NCHUNK], ones_128[:], warm_rhs[:],
                             start=True, stop=True)

        # head-weight tiles: one [128, 8*1024] tile per head, 3 in flight
        wh_pool = ctx.enter_context(tc.tile_pool(name="wh", bufs=3))

        with ExitStack() as actx:
            a_pool = actx.enter_context(tc.tile_pool(name="stageA", bufs=1))
            # xe[half][128, 4*512], we[wave][half][128, 4*512]
            xe = [a_pool.tile([P, (KT_E // 2) * B_LOC], MM, tag=f"xe{i}", name=f"xe{i}")
                  for i in range(2)]
            we = [[a_pool.tile([P, (KT_E // 2) * HALF], MM, tag=f"we{w}{i}", name=f"we{w}{i}")
                   for i in range(2)] for w in range(2)]
            xd = a_pool.tile([P, KT_D * B_LOC], MM, tag="xd", name="xd")
            wd = a_pool.tile([P, KT_D * HID], MM, tag="wd", name="wd")

            # ---- DMA issues, by queue and need-time priority. Each queue
            # sustains ~120 GB/s aggregate; early W_heads tiles are split in
            # two 1 MB halves on different queues so each lands in ~8 us.
            # Pool-blocked wh DMAs (h>=3, which wait for an earlier head's
            # buffer) go ONLY on non-compute queues (sync/gpsimd): on the
            # scalar queue they would deadlock against the relus that
            # retire the earlier head.
            WHH = (KT_H // 2) * HID   # half a head-weight tile (k=0..3)
            wh_tiles = [wh_pool.tile([P, KT_H * HID], MM, tag="whs", name=f"wh{h}")
                        for h in range(HEADS)]
            # sync queue (no compute): stage-A weights, early wh halves,
            # then the pool-gated late heads
            nc.sync.dma_start(we[0][0][:], we_r[0, 0])
            nc.sync.dma_start(we[1][0][:], we_r[1, 0])
            nc.sync.dma_start(we[1][1][:], we_r[1, 1])
            nc.sync.dma_start(wh_tiles[0][:, :WHH], wh_r[0, :, :WHH])
            nc.sync.dma_start(bd_bc[:], b_dec_bc[:])
            nc.sync.dma_start(bh_bc[0][:], b_heads_bc[0])
            nc.sync.dma_start(wh_tiles[1][:, :WHH], wh_r[1, :, :WHH])
            nc.sync.dma_start(bh_bc[1][:], b_heads_bc[1])
            nc.sync.dma_start(wh_tiles[2][:, :WHH], wh_r[2, :, :WHH])
            for h in (3, 5, 7):
                nc.sync.dma_start(wh_tiles[h][:, :WHH], wh_r[h, :, :WHH])
                nc.sync.dma_start(wh_tiles[h][:, WHH:], wh_r[h, :, WHH:])
            # scalar queue: only the enc inputs (no wh traffic: anything that
            # could wait on a pool buffer would deadlock against the relus
            # that retire it, and the scheduler orders conservatively)
            nc.scalar.dma_start(xe[0][:], xe_r[0])
            nc.scalar.dma_start(xe[1][:], xe_r[1])
            nc.scalar.dma_start(benc[:], b_enc_pp[:])
            # gpsimd queue (no compute): dec inputs, early wh halves, bias
            # tiles, late heads
            nc.gpsimd.dma_start(we[0][1][:], we_r[0, 1])
            nc.gpsimd.dma_start(xd[:], xd_r[:])
            nc.gpsimd.dma_start(wd[:], wd_r[:])
            nc.gpsimd.dma_start(wh_tiles[0][:, WHH:], wh_r[0, :, WHH:])
            nc.gpsimd.dma_start(wh_tiles[1][:, WHH:], wh_r[1, :, WHH:])
            nc.gpsimd.dma_start(wh_tiles[2][:, WHH:], wh_r[2, :, WHH:])
            for h in range(2, HEADS):
                nc.gpsimd.dma_start(bh_bc[h][:], b_heads_bc[h])
            for h in (4, 6):
                nc.gpsimd.dma_start(wh_tiles[h][:, :WHH], wh_r[h, :, :WHH])
                nc.gpsimd.dma_start(wh_tiles[h][:, WHH:], wh_r[h, :, WHH:])

            # ---- Stage C tile: hybrid bias (n=0 DVE tt-add, n=1 bias MM) --
            def stage_c_tile(b):
                ps = psums.tile([P, HID], f32, tag="mm", name="ps")
                for k in range(KT_D):
                    nc.tensor.matmul(
                        ps[:, :NCHUNK],
                        xd[:, k * B_LOC + b * P:k * B_LOC + (b + 1) * P],
                        wd[:, k * HID:k * HID + NCHUNK],
                        start=(k == 0), stop=(k == KT_D - 1))
                nc.tensor.matmul(ps[:, NCHUNK:], ones_128[:], bd_bc[:, NCHUNK:],
                                 start=True, stop=False)
                for k in range(KT_D):
                    nc.tensor.matmul(
                        ps[:, NCHUNK:],
                        xd[:, k * B_LOC + b * P:k * B_LOC + (b + 1) * P],
                        wd[:, k * HID + NCHUNK:(k + 1) * HID],
                        start=False, stop=(k == KT_D - 1))
                nc.vector.tensor_tensor(dec_bm[b][:, :NCHUNK], ps[:, :NCHUNK],
                                        bd_bc[:, :NCHUNK], op=add)
                nc.scalar.activation(dec_bm[b][:, :NCHUNK],
                                     dec_bm[b][:, :NCHUNK], Relu)
                nc.scalar.activation(dec_bm[b][:, NCHUNK:], ps[:, NCHUNK:], Relu)

            # ---- Stage A (enc trunk, feature-major), k-outer in 2 waves --
            for wave in range(2):
                pss = [psums.tile([P, HID], f32, tag="mm", name="ps")
                       for _ in range(MT // 2)]
                for k in range(KT_E):
                    xek = xe[k // 4][:, (k % 4) * B_LOC:(k % 4 + 1) * B_LOC]
                    wek = we[wave][k // 4][:, (k % 4) * HALF:(k % 4 + 1) * HALF]
                    for j in range(MT // 2):
                        nc.tensor.matmul(pss[j][:, :B_LOC],
                                         wek[:, j * P:(j + 1) * P], xek,
                                         start=(k == 0), stop=(k == KT_E - 1))
                for j in range(MT // 2):
                    m = wave * (MT // 2) + j
                    nc.scalar.activation(ench[m][:], pss[j][:, :B_LOC], Relu,
                                         bias=benc[:, m:m + 1], scale=1.0)
            for b in range(BT):
                stage_c_tile(b)

        # ---- Stage B + D: heads (batch-major), streaming softmax ----
        head_pool = ctx.enter_context(tc.tile_pool(name="head", bufs=3))
        scratch = ctx.enter_context(tc.tile_pool(name="scratch", bufs=4))

        for h in range(HEADS):
            wh = wh_tiles[h]
            last = h == HEADS - 1
            for b in range(BT):
                head_t = head_pool.tile([P, HID], ST, tag=f"head{b}", name=f"head{b}")
                ps = psums.tile([P, HID], f32, tag="mm", name="ps")
                # n=0 chunk: plain matmuls, bias added by DVE afterwards
                for k in range(KT_H):
                    nc.tensor.matmul(
                        ps[:, :NCHUNK], ench[k][:, b * P:(b + 1) * P],
                        wh[:, k * HID:k * HID + NCHUNK],
                        start=(k == 0), stop=(k == KT_H - 1))
                # n=1 chunk: bias injected by a K=128 matmul (ones/128 against
                # the broadcast-bias tile), relu straight from PSUM
                nc.tensor.matmul(ps[:, NCHUNK:], ones_128[:],
                                 bh_bc[h][:, NCHUNK:], start=True, stop=False)
                for k in range(KT_H):
                    nc.tensor.matmul(
                        ps[:, NCHUNK:], ench[k][:, b * P:(b + 1) * P],
                        wh[:, k * HID + NCHUNK:(k + 1) * HID],
                        start=False, stop=(k == KT_H - 1))
                prod = scratch.tile([P, HID], ST, tag="prod", name="prod")
                s_col = scratch.tile([P, 1], f32, tag="scol", name="scol")
                # n=0: tt-add + in-place relu; n=1: relu from psum
                nc.vector.tensor_tensor(head_t[:, :NCHUNK], ps[:, :NCHUNK],
                                        bh_bc[h][:, :NCHUNK], op=add)
                nc.scalar.activation(head_t[:, :NCHUNK], head_t[:, :NCHUNK],
                                     Relu)
                nc.scalar.activation(head_t[:, NCHUNK:], ps[:, NCHUNK:], Relu)
                # score: s_col = sum_hid(head * dec)  (fused mult+accum)
                if not last:
                    nc.vector.scalar_tensor_tensor(
                        prod[:], head_t[:], 1.0, dec_bm[b][:],
                        op0=mult, op1=mult, accum_out=s_col[:])
                else:
                    # last head: half-tile ops so the kernel tail pipelines
                    s_half = scratch.tile([P, 1], f32, tag="shalf", name="shalf")
                    nc.vector.scalar_tensor_tensor(
                        prod[:, :NCHUNK], head_t[:, :NCHUNK], 1.0,
                        dec_bm[b][:, :NCHUNK], op0=mult, op1=mult,
                        accum_out=s_half[:])
                    nc.vector.scalar_tensor_tensor(
                        prod[:, NCHUNK:], head_t[:, NCHUNK:], 1.0,
                        dec_bm[b][:, NCHUNK:], op0=mult, op1=mult,
                        accum_out=s_col[:])
                    nc.vector.tensor_add(s_col[:], s_col[:], s_half[:])
                # e = exp(score - C)
                nc.scalar.activation(e_all[b][:, h:h + 1], s_col[:], Exp,
                                     bias=negC[:], scale=1.0)
                # out_acc += e_h * head (DVE fused stt, in-place accumulate);
                # h==0 initializes via tensor_scalar (no memset, no add)
                if h == 0:
                    nc.vector.tensor_scalar(
                        out_acc[b][:], head_t[:], e_all[b][:, h:h + 1], None,
                        op0=mult)
                elif not last:
                    nc.vector.scalar_tensor_tensor(
                        out_acc[b][:], head_t[:], e_all[b][:, h:h + 1],
                        out_acc[b][:], op0=mult, op1=add)
                else:
                    for n in range(NC_H):
                        ncol = slice(n * NCHUNK, (n + 1) * NCHUNK)
                        nc.vector.scalar_tensor_tensor(
                            out_acc[b][:, ncol], head_t[:, ncol],
                            e_all[b][:, h:h + 1],
                            out_acc[b][:, ncol], op0=mult, op1=add)

        # ---- Final: divide by sum of exps, write out ----
        fin = ctx.enter_context(tc.tile_pool(name="fin", bufs=2))
        for b in range(BT):
            s_sum = fin.tile([P, 1], f32, tag="ssum", name="ssum")
            rinv = fin.tile([P, 1], f32, tag="rinv", name="rinv")
            nc.vector.reduce_sum(s_sum[:], e_all[b][:], axis=X)
            nc.vector.reciprocal(rinv[:], s_sum[:])
            out_f = fin.tile([P, HID], f32, tag="outf", name="outf")
            nc.vector.tensor_scalar_mul(out_f[:], out_acc[b][:], rinv[:])
            nc.sync.dma_start(out_d[b * P:(b + 1) * P, :], out_f[:])

    nc.compile()
    return nc


def _get_nc():
    if "nc" not in _cache:
        _cache["nc"] = _build()
    return _cache["nc"]


def build_in_maps(encoder_input, decoder_input, W_enc, b_enc, W_heads,
                  b_heads, W_dec, b_dec):
    import ml_dtypes
    bf = ml_dtypes.bfloat16
    f32c = lambda a: np.asarray(a, dtype=np.float32)
    cast = lambda a: np.ascontiguousarray(a, dtype=np.float32).astype(bf)

    KT_E, KT_D, KT_H = ENC_DIM // P, DEC_DIM // P, HID // P

    xeT = f32c(encoder_input).T                     # [1024, 4096]
    xdT = f32c(decoder_input).T                     # [512, 4096]
    W_enc = f32c(W_enc)                             # [1024, 1024]
    W_dec = f32c(W_dec)                             # [512, 1024]
    W_heads = f32c(W_heads)                         # [8, 1024, 1024]

    # w_enc_r[w, i][p, k4*512 + c] = W_enc[(i*4+k4)*128+p, w*512 + c]
    we4 = W_enc.reshape(KT_E, P, 2, HID // 2)       # [k, p, wave, c]
    we_r = np.zeros((2, 2, P, (KT_E // 2) * (HID // 2)), np.float32)
    for w in range(2):
        for i in range(2):
            blk = we4[i * 4:(i + 1) * 4, :, w, :]   # [4k, 128, 512]
            we_r[w, i] = blk.transpose(1, 0, 2).reshape(P, -1)
    xe4 = xeT.reshape(KT_E, P, BATCH)
    wd4 = W_dec.reshape(KT_D, P, HID)
    wd_r = np.ascontiguousarray(wd4.transpose(1, 0, 2).reshape(P, -1))
    wh4 = W_heads.reshape(HEADS, KT_H, P, HID)
    wh_r = np.ascontiguousarray(wh4.transpose(0, 2, 1, 3).reshape(HEADS, P, -1))

    bh_bc = np.broadcast_to(f32c(b_heads)[:, None, :], (HEADS, P, HID))
    bd_bc = np.broadcast_to(f32c(b_dec)[None, :], (P, HID))
    shared = {
        "w_enc_r": cast(we_r),
        "b_enc_pp": np.ascontiguousarray(f32c(b_enc).reshape(HID // P, P).T),
        "w_heads_r": cast(wh_r),
        "b_heads_bc": cast(bh_bc),
        "w_dec_r": cast(wd_r),
        "b_dec_bc": cast(bd_bc),
    }
    xd4 = xdT.reshape(KT_D, P, BATCH)
    in_maps = []
    for c in range(N_CORES):
        sl = slice(c * B_LOC, (c + 1) * B_LOC)
        m = dict(shared)
        xe_c = xe4[:, :, sl]                        # [8k, 128, 512]
        m["x_enc_r"] = cast(np.stack(
            [xe_c[i * 4:(i + 1) * 4].transpose(1, 0, 2).reshape(P, -1)
             for i in range(2)]))
        m["x_dec_r"] = cast(xd4[:, :, sl].transpose(1, 0, 2).reshape(P, -1))
        in_maps.append(m)
    return in_maps


def kernel(encoder_input, decoder_input, W_enc, b_enc, W_heads, b_heads,
           W_dec, b_dec):
    from concourse.bass_utils import run_bass_kernel_spmd

    nc = _get_nc()
    in_maps = build_in_maps(encoder_input, decoder_input, W_enc, b_enc,
                            W_heads, b_heads, W_dec, b_dec)
    res = run_bass_kernel_spmd(nc, in_maps, list(range(N_CORES)))
    out = np.concatenate([res.results[c]["out"] for c in range(N_CORES)], axis=0)
    return out.astype(np.float32)


# revision 21
# speedup vs baseline: 1.1179x; 1.0378x over previous
"""Trainium2 Bass kernel for nn_Attention2 (8-head encoder/decoder attention mix).

Reference computation (full batch B=4096):
    enc_h  = relu(encoder_input @ W_enc + b_enc)               [B, 1024]
    heads  = relu(einsum('bh,khd->kbd', enc_h, W_heads) + b_heads)  [8, B, 1024]
    dec_H  = relu(decoder_input @ W_dec + b_dec)               [B, 1024]
    scores = sum(heads * dec_H, axis=2)                        [8, B]
    attn   = softmax(scores.T, axis=1)                         [B, 8]
    out    = einsum('kbd,bk->bd', heads, attn)                 [B, 1024]

Sharding: pure data-parallel over batch across 8 NeuronCores (B_loc = 512
per core, params replicated, zero collectives).

v5 design (evolved from the v1 all-bias-matmul kernel via trace analysis):
  - Half the bias-injection matmuls removed from the PE stream (644 MMs:
    608 real + 36 bias vs v1's 680). Per [128,1024] output tile, the n=0
    chunk gets its bias from a DVE tensor_tensor add (PSUM + broadcast-bias
    SBUF tile) and the n=1 chunk from a K=128 matmul of ones/128 against
    the same broadcast tile (then relu straight from PSUM). This hybrid
    keeps BOTH engines under the PE budget per head-batch-tile
    (PE 17 MM = 3.67 us; DVE = tt-add 690 + score stt 1224 + out tt-add
    690 = 2.6 us; ScalarE = 2 relu + exp + scale-copy = 2.5 us) - the
    full-DVE-bias variant measured DVE == PE and drained a ~20 us tail.
  - out_acc path: head_s = ACT(head, Copy, scale=e_h) on ScalarE (bf16),
    out_acc += head_s via all-bf16 tensor_tensor add (2x DVE mode, 690 ns
    vs 1226 stt). h==0 initializes out_acc directly (no memsets).
  - Host repacks weights so every big load is one contiguous DMA:
    W_heads -> [H][128, 8*1024] (one 2 MB DMA per head, 16 KB rows),
    W_enc -> wave-split quarters, x_encT halves, x_dec/W_dec one DMA each.
    ~28 DMA issues total, spread over the sync/scalar/gpsimd queues by
    need-time priority.
  - Stage C (dec) b-tiles are interleaved between the two stage-A waves so
    the PE has work while the second half of the stage-A inputs lands.
  - ~10 warmup matmuls on constants right after the preamble warm the HAM
    clock gate (cold PE runs 1.2 GHz for its first ~3.4 us) during the
    initial DMA wait.
  - Streaming normalizer-free softmax: e = exp(score - 24) (scores
    measured in [14, 34]); divide by sum(e) at the end.

Measured v4 milestones (FAST clock regime): MM stream at the 216 ns
roofline spacing, first MM at 7.5 us.
"""

import os
import numpy as np
from contextlib import ExitStack

N_CORES = 8
ENC_DIM, DEC_DIM, HID, HEADS, BATCH = 1024, 512, 1024, 8, 4096
B_LOC = BATCH // N_CORES          # 512 batch rows per core
P = 128                           # SBUF partitions
NCHUNK = 512                      # matmul moving free-dim (one PSUM bank)
SCORE_SHIFT = 24.0                # scores measured in [14.2, 34.0]

_cache = {}


def _build():
    import concourse.tile as tile
    from concourse import bacc, mybir

    f32 = mybir.dt.float32
    bf16 = mybir.dt.bfloat16
    MM = bf16
    ST = f32                      # head storage dtype (score stt is f32-fast)
    Relu = mybir.ActivationFunctionType.Relu
    Exp = mybir.ActivationFunctionType.Exp
    Copy = mybir.ActivationFunctionType.Copy
    X = mybir.AxisListType.X
    mult = mybir.AluOpType.mult
    add = mybir.AluOpType.add

    KT_E = ENC_DIM // P           # 8 contraction tiles (enc dim)
    KT_H = HID // P               # 8 contraction tiles (hid dim)
    KT_D = DEC_DIM // P           # 4 contraction tiles (dec dim)
    MT = HID // P                 # 8 hid tiles (feature-major partitions)
    BT = B_LOC // P               # 4 batch tiles
    NC_H = HID // NCHUNK          # 2 moving chunks over hid
    HALF = HID // 2               # 512

    N_WARMUP = int(os.environ.get("BASS_WARMUP", "10"))

    nc = bacc.Bacc("TRN2", target_bir_lowering=False, debug=False,
                   num_devices=N_CORES)

    # host-repacked inputs (see build_in_maps)
    xe_r = nc.dram_tensor("x_enc_r", [2, P, (KT_E // 2) * B_LOC], MM,
                          kind="ExternalInput").ap()
    we_r = nc.dram_tensor("w_enc_r", [2, 2, P, (KT_E // 2) * HALF], MM,
                          kind="ExternalInput").ap()
    xd_r = nc.dram_tensor("x_dec_r", [P, KT_D * B_LOC], MM,
                          kind="ExternalInput").ap()
    wd_r = nc.dram_tensor("w_dec_r", [P, KT_D * HID], MM,
                          kind="ExternalInput").ap()
    wh_r = nc.dram_tensor("w_heads_r", [HEADS, P, KT_H * HID], MM,
                          kind="ExternalInput").ap()
    b_enc_pp = nc.dram_tensor("b_enc_pp", [P, MT], f32, kind="ExternalInput").ap()
    # broadcast bias tiles: bias replicated across the 128 partitions
    b_heads_bc = nc.dram_tensor("b_heads_bc", [HEADS, P, HID], MM,
                                kind="ExternalInput").ap()
    b_dec_bc = nc.dram_tensor("b_dec_bc", [P, HID], MM, kind="ExternalInput").ap()
    out_d = nc.dram_tensor("out", [B_LOC, HID], f32, kind="ExternalOutput").ap()

    with tile.TileContext(nc) as tc, ExitStack() as ctx:
        persist = ctx.enter_context(tc.tile_pool(name="persist", bufs=1))
        psums = ctx.enter_context(tc.tile_pool(name="psums", bufs=4, space="PSUM"))

        # --- constants / biases ---
        # ones/128 so a K=128 matmul against the full broadcast-bias tile
        # sums to exactly the bias
        ones_128 = persist.tile([P, P], MM, tag="ones128", name="ones128")
        nc.vector.memset(ones_128[:], 1.0 / P)
        warm_rhs = persist.tile([P, NCHUNK], MM, tag="wrhs", name="wrhs")
        nc.vector.memset(warm_rhs[:], 0.5)
        negC = persist.tile([P, 1], f32, tag="negC", name="negC")
        nc.vector.memset(negC[:], -SCORE_SHIFT)
        benc = persist.tile([P, MT], f32, tag="benc", name="benc")
        bh_bc = [persist.tile([P, HID], MM, tag=f"bhb{h}", name=f"bhb{h}")
                 for h in range(HEADS)]
        bd_bc = persist.tile([P, HID], MM, tag="bdb", name="bdb")

        # --- persistent activations ---
        ench = [persist.tile([P, B_LOC], MM, tag=f"ench{m}", name=f"ench{m}") for m in range(MT)]
        dec_bm = [persist.tile([P, HID], ST, tag=f"dec{b}", name=f"dec{b}") for b in range(BT)]
        e_all = [persist.tile([P, HEADS], f32, tag=f"eall{b}", name=f"eall{b}") for b in range(BT)]
        out_acc = [persist.tile([P, HID], ST, tag=f"oacc{b}", name=f"oacc{b}") for b in range(BT)]

        # ---- PE warmup: matmuls on constants so HAM un-throttles while the
        # first input DMAs are still in flight. Output never read.
        warm_ps = psums.tile([P, HID], f32, tag="mm", name="warm")
        for _ in range(N_WARMUP):
            nc.tensor.matmul(warm_ps[:, :NCHUNK], ones_128[:], warm_rhs[:],
                             start=True, stop=True)

        # head-weight tiles: one [128, 8*1024] tile per head, 3 in flight
        wh_pool = ctx.enter_context(tc.tile_pool(name="wh", bufs=4))

        with ExitStack() as actx:
            a_pool = actx.enter_context(tc.tile_pool(name="stageA", bufs=1))
            # xe[half][128, 4*512], we[wave][half][128, 4*512]
            xe = [a_pool.tile([P, (KT_E // 2) * B_LOC], MM, tag=f"xe{i}", name=f"xe{i}")
                  for i in range(2)]
            we = [[a_pool.tile([P, (KT_E // 2) * HALF], MM, tag=f"we{w}{i}", name=f"we{w}{i}")
                   for i in range(2)] for w in range(2)]
            xd = a_pool.tile([P, KT_D * B_LOC], MM, tag="xd", name="xd")
            wd = a_pool.tile([P, KT_D * HID], MM, tag="wd", name="wd")

            # ---- DMA issues, by queue and need-time priority. Each queue
            # sustains ~120 GB/s aggregate; early W_heads tiles are split in
            # two 1 MB halves on different queues so each lands in ~8 us.
            # Pool-blocked wh DMAs (h>=3, which wait for an earlier head's
            # buffer) go ONLY on non-compute queues (sync/gpsimd): on the
            # scalar queue they would deadlock against the relus that
            # retire the earlier head.
            WHH = (KT_H // 2) * HID   # half a head-weight tile (k=0..3)
            wh_tiles = [wh_pool.tile([P, KT_H * HID], MM, tag="whs", name=f"wh{h}")
                        for h in range(HEADS)]
            # sync queue (no compute): stage-A weights, a-halves of every
            # head, small biases. Each queue moves ~0.1 MB/us; one 2 MB head
            # per period (14.7 us) needs both queues carrying 1 MB each.
            nc.sync.dma_start(we[0][0][:], we_r[0, 0])
            nc.sync.dma_start(we[1][0][:], we_r[1, 0])
            nc.sync.dma_start(we[1][1][:], we_r[1, 1])
            nc.sync.dma_start(wh_tiles[0][:, :WHH], wh_r[0, :, :WHH])
            nc.sync.dma_start(bd_bc[:], b_dec_bc[:])
            nc.sync.dma_start(bh_bc[0][:], b_heads_bc[0])
            nc.sync.dma_start(wh_tiles[1][:, :WHH], wh_r[1, :, :WHH])
            nc.sync.dma_start(bh_bc[1][:], b_heads_bc[1])
            for h in range(2, HEADS):
                nc.sync.dma_start(wh_tiles[h][:, :WHH], wh_r[h, :, :WHH])
            # scalar queue: only never-blocking DMAs (pool-gated wh DMAs on a
            # compute queue would deadlock against the relus that retire the
            # earlier head)
            nc.scalar.dma_start(xe[0][:], xe_r[0])
            nc.scalar.dma_start(xe[1][:], xe_r[1])
            nc.scalar.dma_start(benc[:], b_enc_pp[:])
            # gpsimd queue (no compute): dec inputs, b-halves, bias tiles
            nc.gpsimd.dma_start(we[0][1][:], we_r[0, 1])
            nc.gpsimd.dma_start(xd[:], xd_r[:])
            nc.gpsimd.dma_start(wd[:], wd_r[:])
            nc.gpsimd.dma_start(wh_tiles[0][:, WHH:], wh_r[0, :, WHH:])
            nc.gpsimd.dma_start(wh_tiles[1][:, WHH:], wh_r[1, :, WHH:])
            for h in range(2, 5):
                nc.gpsimd.dma_start(bh_bc[h][:], b_heads_bc[h])
            nc.gpsimd.dma_start(wh_tiles[2][:, WHH:], wh_r[2, :, WHH:])
            for h in range(5, HEADS):
                nc.gpsimd.dma_start(bh_bc[h][:], b_heads_bc[h])
            for h in range(3, HEADS):
                nc.gpsimd.dma_start(wh_tiles[h][:, WHH:], wh_r[h, :, WHH:])

            # ---- Stage C tile: hybrid bias (n=0 DVE tt-add, n=1 bias MM) --
            def stage_c_tile(b):
                ps = psums.tile([P, HID], f32, tag="mm", name="ps")
                for k in range(KT_D):
                    nc.tensor.matmul(
                        ps[:, :NCHUNK],
                        xd[:, k * B_LOC + b * P:k * B_LOC + (b + 1) * P],
                        wd[:, k * HID:k * HID + NCHUNK],
                        start=(k == 0), stop=(k == KT_D - 1))
                nc.tensor.matmul(ps[:, NCHUNK:], ones_128[:], bd_bc[:, NCHUNK:],
                                 start=True, stop=False)
                for k in range(KT_D):
                    nc.tensor.matmul(
                        ps[:, NCHUNK:],
                        xd[:, k * B_LOC + b * P:k * B_LOC + (b + 1) * P],
                        wd[:, k * HID + NCHUNK:(k + 1) * HID],
                        start=False, stop=(k == KT_D - 1))
                nc.vector.tensor_tensor(dec_bm[b][:, :NCHUNK], ps[:, :NCHUNK],
                                        bd_bc[:, :NCHUNK], op=add)
                nc.scalar.activation(dec_bm[b][:, :NCHUNK],
                                     dec_bm[b][:, :NCHUNK], Relu)
                nc.scalar.activation(dec_bm[b][:, NCHUNK:], ps[:, NCHUNK:], Relu)

            # ---- Stage A (enc trunk, feature-major), k-outer in 2 waves --
            for wave in range(2):
                pss = [psums.tile([P, HID], f32, tag="mm", name="ps")
                       for _ in range(MT // 2)]
                for k in range(KT_E):
                    xek = xe[k // 4][:, (k % 4) * B_LOC:(k % 4 + 1) * B_LOC]
                    wek = we[wave][k // 4][:, (k % 4) * HALF:(k % 4 + 1) * HALF]
                    for j in range(MT // 2):
                        nc.tensor.matmul(pss[j][:, :B_LOC],
                                         wek[:, j * P:(j + 1) * P], xek,
                                         start=(k == 0), stop=(k == KT_E - 1))
                for j in range(MT // 2):
                    m = wave * (MT // 2) + j
                    nc.scalar.activation(ench[m][:], pss[j][:, :B_LOC], Relu,
                                         bias=benc[:, m:m + 1], scale=1.0)
            for b in range(BT):
                stage_c_tile(b)

        # ---- Stage B + D: heads (batch-major), streaming softmax ----
        head_pool = ctx.enter_context(tc.tile_pool(name="head", bufs=3))
        scratch = ctx.enter_context(tc.tile_pool(name="scratch", bufs=4))

        for h in range(HEADS):
            wh = wh_tiles[h]
            for b in range(BT):
                last = h == HEADS - 1 and b == BT - 1
                head_t = head_pool.tile([P, HID], ST, tag=f"head{b}", name=f"head{b}")
                ps = psums.tile([P, HID], f32, tag="mm", name="ps")
                # n=0 chunk: plain matmuls, bias added by DVE afterwards
                for k in range(KT_H):
                    nc.tensor.matmul(
                        ps[:, :NCHUNK], ench[k][:, b * P:(b + 1) * P],
                        wh[:, k * HID:k * HID + NCHUNK],
                        start=(k == 0), stop=(k == KT_H - 1))
                # n=1 chunk: bias injected by a K=128 matmul (ones/128 against
                # the broadcast-bias tile), relu straight from PSUM
                nc.tensor.matmul(ps[:, NCHUNK:], ones_128[:],
                                 bh_bc[h][:, NCHUNK:], start=True, stop=False)
                for k in range(KT_H):
                    nc.tensor.matmul(
                        ps[:, NCHUNK:], ench[k][:, b * P:(b + 1) * P],
                        wh[:, k * HID + NCHUNK:(k + 1) * HID],
                        start=False, stop=(k == KT_H - 1))
                prod = scratch.tile([P, HID], ST, tag="prod", name="prod")
                s_col = scratch.tile([P, 1], f32, tag="scol", name="scol")
                # n=0: tt-add + in-place relu; n=1: relu from psum
                nc.vector.tensor_tensor(head_t[:, :NCHUNK], ps[:, :NCHUNK],
                                        bh_bc[h][:, :NCHUNK], op=add)
                nc.scalar.activation(head_t[:, :NCHUNK], head_t[:, :NCHUNK],
                                     Relu)
                nc.scalar.activation(head_t[:, NCHUNK:], ps[:, NCHUNK:], Relu)
                # score: s_col = sum_hid(head * dec)  (fused mult+accum)
                if not last:
                    nc.vector.scalar_tensor_tensor(
                        prod[:], head_t[:], 1.0, dec_bm[b][:],
                        op0=mult, op1=mult, accum_out=s_col[:])
                else:
                    # last head: half-tile ops so the kernel tail pipelines
                    s_half = scratch.tile([P, 1], f32, tag="shalf", name="shalf")
                    nc.vector.scalar_tensor_tensor(
                        prod[:, :NCHUNK], head_t[:, :NCHUNK], 1.0,
                        dec_bm[b][:, :NCHUNK], op0=mult, op1=mult,
                        accum_out=s_half[:])
                    nc.vector.scalar_tensor_tensor(
                        prod[:, NCHUNK:], head_t[:, NCHUNK:], 1.0,
                        dec_bm[b][:, NCHUNK:], op0=mult, op1=mult,
                        accum_out=s_col[:])
                    nc.vector.tensor_add(s_col[:], s_col[:], s_half[:])
                # e = exp(score - C)
                nc.scalar.activation(e_all[b][:, h:h + 1], s_col[:], Exp,
                                     bias=negC[:], scale=1.0)
                # out_acc += e_h * head (DVE fused stt, in-place accumulate);
                # h==0 initializes via tensor_scalar (no memset, no add)
                if h == 0:
                    nc.vector.tensor_scalar(
                        out_acc[b][:], head_t[:], e_all[b][:, h:h + 1], None,
                        op0=mult)
                elif not last:
                    nc.vector.scalar_tensor_tensor(
                        out_acc[b][:], head_t[:], e_all[b][:, h:h + 1],
                        out_acc[b][:], op0=mult, op1=add)
                else:
                    nc.vector.scalar_tensor_tensor(
                        out_acc[b][:], head_t[:], e_all[b][:, h:h + 1],
                        out_acc[b][:], op0=mult, op1=add)

        # ---- Final: divide by sum of exps, write out ----
        fin = ctx.enter_context(tc.tile_pool(name="fin", bufs=2))
        for b in range(BT):
            s_sum = fin.tile([P, 1], f32, tag="ssum", name="ssum")
            rinv = fin.tile([P, 1], f32, tag="rinv", name="rinv")
            nc.vector.reduce_sum(s_sum[:], e_all[b][:], axis=X)
            nc.vector.reciprocal(rinv[:], s_sum[:])
            out_f = fin.tile([P, HID], f32, tag="outf", name="outf")
            nc.scalar.activation(out_f[:], out_acc[b][:], Copy, scale=rinv[:])
            nc.sync.dma_start(out_d[b * P:(b + 1) * P, :], out_f[:])

    nc.compile()
    return nc


def _get_nc():
    if "nc" not in _cache:
        _cache["nc"] = _build()
    return _cache["nc"]


def build_in_maps(encoder_input, decoder_input, W_enc, b_enc, W_heads,
                  b_heads, W_dec, b_dec):
    import ml_dtypes
    bf = ml_dtypes.bfloat16
    f32c = lambda a: np.asarray(a, dtype=np.float32)
    cast = lambda a: np.ascontiguousarray(a, dtype=np.float32).astype(bf)

    KT_E, KT_D, KT_H = ENC_DIM // P, DEC_DIM // P, HID // P

    xeT = f32c(encoder_input).T                     # [1024, 4096]
    xdT = f32c(decoder_input).T                     # [512, 4096]
    W_enc = f32c(W_enc)                             # [1024, 1024]
    W_dec = f32c(W_dec)                             # [512, 1024]
    W_heads = f32c(W_heads)                         # [8, 1024, 1024]

    # w_enc_r[w, i][p, k4*512 + c] = W_enc[(i*4+k4)*128+p, w*512 + c]
    we4 = W_enc.reshape(KT_E, P, 2, HID // 2)       # [k, p, wave, c]
    we_r = np.zeros((2, 2, P, (KT_E // 2) * (HID // 2)), np.float32)
    for w in range(2):
        for i in range(2):
            blk = we4[i * 4:(i + 1) * 4, :, w, :]   # [4k, 128, 512]
            we_r[w, i] = blk.transpose(1, 0, 2).reshape(P, -1)
    xe4 = xeT.reshape(KT_E, P, BATCH)
    wd4 = W_dec.reshape(KT_D, P, HID)
    wd_r = np.ascontiguousarray(wd4.transpose(1, 0, 2).reshape(P, -1))
    wh4 = W_heads.reshape(HEADS, KT_H, P, HID)
    wh_r = np.ascontiguousarray(wh4.transpose(0, 2, 1, 3).reshape(HEADS, P, -1))

    bh_bc = np.broadcast_to(f32c(b_heads)[:, None, :], (HEADS, P, HID))
    bd_bc = np.broadcast_to(f32c(b_dec)[None, :], (P, HID))
    shared = {
        "w_enc_r": cast(we_r),
        "b_enc_pp": np.ascontiguousarray(f32c(b_enc).reshape(HID // P, P).T),
        "w_heads_r": cast(wh_r),
        "b_heads_bc": cast(bh_bc),
        "w_dec_r": cast(wd_r),
        "b_dec_bc": cast(bd_bc),
    }
    xd4 = xdT.reshape(KT_D, P, BATCH)
    in_maps = []
    for c in range(N_CORES):
        sl = slice(c * B_LOC, (c + 1) * B_LOC)
        m = dict(shared)
        xe_c = xe4[:, :, sl]                        # [8k, 128, 512]
        m["x_enc_r"] = cast(np.stack(
            [xe_c[i * 4:(i + 1) * 4].transpose(1, 0, 2).reshape(P, -1)
             for i in range(2)]))
        m["x_dec_r"] = cast(xd4[:, :, sl].transpose(1, 0, 2).reshape(P, -1))
        in_maps.append(m)
    return in_maps


def kernel(encoder_input, decoder_input, W_enc, b_enc, W_heads, b_heads,
           W_dec, b_dec):
    from concourse.bass_utils import run_bass_kernel_spmd

    nc = _get_nc()
    in_maps = build_in_maps(encoder_input, decoder_input, W_enc, b_enc,
                            W_heads, b_heads, W_dec, b_dec)
    res = run_bass_kernel_spmd(nc, in_maps, list(range(N_CORES)))
    out = np.concatenate([res.results[c]["out"] for c in range(N_CORES)], axis=0)
    return out.astype(np.float32)
